# revision 28
# baseline (speedup 1.0000x reference)
"""Trainium2 Bass kernel for nn_CNNCrossPatchBackbone (sparse cross-patch attention).

Strategy: 8 cores = 4 batches x {ctx self-attention, tgt cross-attention}.
Each core runs an identical-shape problem: 1024 q-tokens x 1024 kv-tokens,
16 heads of dim 64, D=1024. Fully task-parallel, no collectives.

Host side does all elementwise/index prep: stable argsort, token gather, the
full 2D-RoPE rotation (exact fp32 mirror of the reference), the d-major
transpose of x, weight transposes packed [P, DT*cols] (so each tensor loads
in a few large DMAs), the 1/sqrt(hd)=2^-3 scale fold into wq/bq (exact), and
a combined output bias b_total = b_o + b_v @ Wo^T added after the gather
(softmax rows sum to 1, so the V-bias passes through attention additively).

Device side is one software-pipelined bf16 PE stream (fp32 PSUM accum):
  * Projections: Q^T/K^T in [dout, tok] layout (bias folded into the DVE
    PSUM->SBUF copy); V in natural [tok, dout] layout with a ones column
    per head (VA tiles memset to 1.0; the proj copies overwrite the data
    columns) so the AV matmul also produces the softmax denominator.
  * Scores: S^T = K_h^T^T Q^T per (head-pair, kv-tile) into [128,1024]
    PSUM; the two K=64 contractions of a head pair target disjoint PE
    row-groups via tile_position, paired by rhs column half. exp on ACT
    (max-subtraction skipped: scores ~N(0,1)); ACT runs (almost) nothing
    else, so its 222-cycle access bubble is paid only 128x on [128,1024]
    activations.
  * AV: O^T-direct - stationary = V_aug [128,65], moving = A^T (N=512),
    PSUM out [65, 512]. Row 64 is the denominator: staged to SBUF
    partition 0 on DVE (the custom-DVE reciprocal misreads PSUM at a
    partition offset - verified on HW), reciprocal_approx_fast, then
    partition-broadcast on the otherwise-idle gpsimd engine, and a DVE
    multiply writes the normalized O^T chunk directly into the single
    [128, 8*1024] O^T tile the output projection consumes.
  * Output projection accumulates y[tok, dout] from O^T chunks; biases on
    host.
  * Scheduling: the in-order PE stream interleaves, at ~0.5-2us
    granularity, each head-pair's 8 S units with the NEXT block's K/Q
    projection chains and the PREVIOUS pair's AV units, so the PE never
    stalls on the exp pipeline and the exp never starves. Inputs arrive
    in consumption-ordered quarter-tensor chunks alternating across both
    HWDGE queues (kv side first, then wv for the V-projection prologue
    fillers, then the q side). The final AV pair runs right before the
    output projection, which streams back per q-tile.
"""

import sys

sys.path.insert(0, "/opt/trn_rl_repo")

import ml_dtypes
import numpy as np

import concourse.bass as bass  # noqa: F401
import concourse.tile as tile
from concourse import bacc, mybir
from concourse.bass_utils import run_bass_kernel_spmd

B, K, D, H = 4, 2048, 1024, 16
NCTX = K // 2
NTOK = 1024  # tokens per side after the ctx/tgt split
HD = D // H  # 64
IMAGE_SIZE = 224.0
MAX_POS = 1024
P = 128
DT = D // P  # 8 d-tiles
TT = NTOK // P  # 8 token-tiles
F32 = mybir.dt.float32
BF16 = mybir.dt.bfloat16
A_BUFS = 26  # a-tile pool: ~2 head-pairs (AV frees a pair only at its last unit)


def build_nc():
    nc = bacc.Bacc("TRN2", target_bir_lowering=False, debug=False, num_devices=8)

    # all per-core tensors are packed [P, DT*cols] so each loads in 1-2 DMAs
    xq_ext = nc.dram_tensor("xqT", [P, DT * NTOK], BF16, kind="ExternalInput")
    xkv_ext = nc.dram_tensor("xkvT", [P, DT * NTOK], BF16, kind="ExternalInput")
    w_ext = nc.dram_tensor("wqkvT", [3, P, DT * D], BF16, kind="ExternalInput")
    wo_ext = nc.dram_tensor("woT", [P, DT * D], BF16, kind="ExternalInput")
    bias_ext = nc.dram_tensor("biasqk", [P, 2 * DT], F32, kind="ExternalInput")
    out_ext = nc.dram_tensor("out", [NTOK, D], F32, kind="ExternalOutput")

    with tile.TileContext(nc) as tc:
        with (
            tc.tile_pool(name="p_bias", bufs=1) as p_bias,
            tc.tile_pool(name="p_qt", bufs=DT) as p_qt,
            tc.tile_pool(name="p_kt", bufs=DT) as p_kt,
            tc.tile_pool(name="p_va", bufs=TT) as p_va,
            tc.tile_pool(name="p_a", bufs=A_BUFS) as p_a,
            tc.tile_pool(name="p_r", bufs=2) as p_r,
            tc.tile_pool(name="p_rb", bufs=2) as p_rb,
            tc.tile_pool(name="p_ot", bufs=1) as p_ot,
            tc.tile_pool(name="p_xkv", bufs=1) as p_xkv,
            tc.tile_pool(name="ps_a", bufs=2, space="PSUM") as ps_a,
            tc.tile_pool(name="ps_s", bufs=2, space="PSUM") as ps_s,
            tc.tile_pool(name="ps_o", bufs=2, space="PSUM") as ps_o,
        ):
            biasqk = p_bias.tile([P, 2 * DT], F32)
            nc.sync.dma_start(biasqk[:], bias_ext.ap())

            QT = [p_qt.tile([P, NTOK], BF16, tag="qt", name=f"qt{i}") for i in range(DT)]
            KT = [p_kt.tile([P, NTOK], BF16, tag="kt", name=f"kt{i}") for i in range(DT)]
            VA = [
                p_va.tile([P, H * (HD + 1)], BF16, tag="va", name=f"va{i}")
                for i in range(TT)
            ]
            OT = p_ot.tile([P, DT * NTOK], BF16, tag="ot")
            XKV = p_xkv.tile([P, DT * NTOK], BF16, tag="xkv")

            # ones for the V denominator columns
            for t in VA:
                nc.gpsimd.memset(t[:], 1.0)

            # ---- everything is software-pipelined into one PE stream ----
            with (
                tc.tile_pool(name="p_wv", bufs=1) as p_wv,
                tc.tile_pool(name="p_wk", bufs=1) as p_wk,
                tc.tile_pool(name="p_wq", bufs=1) as p_wq,
                tc.tile_pool(name="p_xq", bufs=1) as p_xq,
            ):
                WK = p_wk.tile([P, DT * D], BF16, tag="wk")
                WQ = p_wq.tile([P, DT * D], BF16, tag="wq")
                XQ = p_xq.tile([P, DT * NTOK], BF16, tag="xq")
                WV = p_wv.tile([P, DT * D], BF16, tag="wv")
                # consumption-ordered chunks alternating across both HWDGE
                # queues: K inputs, then the q side (Q gates the first exp,
                # so it beats wv - the V units are PE filler with slack).
                CH = DT * NTOK // 4
                groups = (
                    [(XKV, xkv_ext.ap()), (WK, w_ext.ap()[1])],
                    [(XQ, xq_ext.ap()), (WQ, w_ext.ap()[0])],
                    [(WV, w_ext.ap()[2])],
                )
                for pairs in groups:
                    for ci in range(4):
                        cs = slice(ci * CH, (ci + 1) * CH)
                        q = nc.sync if ci % 2 == 0 else nc.scalar
                        for dst, srcap in pairs:
                            q.dma_start(dst[:, cs], srcap[:, cs])

                def emit_v(tt, nh):
                    v_ps = ps_a.tile([P, 512], F32, tag="psa")
                    for dt in range(DT):
                        nc.tensor.matmul(
                            v_ps[:],
                            XKV[:, dt * NTOK + tt * P : dt * NTOK + (tt + 1) * P],
                            WV[:, dt * D + nh * 512 : dt * D + (nh + 1) * 512],
                            start=(dt == 0),
                            stop=(dt == DT - 1),
                        )
                    out_ap = VA[tt][:].rearrange("p (h c) -> p h c", c=HD + 1)[
                        :, nh * 8 : (nh + 1) * 8, 0:HD
                    ]
                    nc.vector.tensor_copy(
                        out_ap, v_ps[:].rearrange("p (h c) -> p h c", c=HD)
                    )

                def emit_k(c, nh):
                    # K^T block c (heads 2c in rows 0:64, 2c+1 in 64:128);
                    # bias folded into the PSUM->SBUF copy
                    ps = ps_a.tile([P, 512], F32, tag="psa")
                    for dt in range(DT):
                        nc.tensor.matmul(
                            ps[:],
                            WK[:, dt * D + c * P : dt * D + (c + 1) * P],
                            XKV[:, dt * NTOK + nh * 512 : dt * NTOK + (nh + 1) * 512],
                            start=(dt == 0),
                            stop=(dt == DT - 1),
                        )
                    nc.vector.tensor_scalar_add(
                        KT[c][:, nh * 512 : (nh + 1) * 512],
                        ps[:],
                        biasqk[:, DT + c : DT + c + 1],
                    )

                def emit_q(c, nh):
                    ps = ps_a.tile([P, 512], F32, tag="psa")
                    for dt in range(DT):
                        nc.tensor.matmul(
                            ps[:],
                            WQ[:, dt * D + c * P : dt * D + (c + 1) * P],
                            XQ[:, dt * NTOK + nh * 512 : dt * NTOK + (nh + 1) * 512],
                            start=(dt == 0),
                            stop=(dt == DT - 1),
                        )
                    nc.vector.tensor_scalar_add(
                        QT[c][:, nh * 512 : (nh + 1) * 512], ps[:], biasqk[:, c : c + 1]
                    )

                def emit_s_unit(c, kc, a_e, a_o):
                    # scores for BOTH heads of pair c against kv-tile kc: the
                    # two K=64 contractions occupy disjoint row-groups of the
                    # PE array (tile_position). Pair the matmuls BY COLUMN
                    # HALF so consecutive instructions stream the same rhs
                    # columns and co-stream through the array on HW.
                    s_e = ps_s.tile([P, NTOK], F32, tag="pss")
                    s_o = ps_s.tile([P, NTOK], F32, tag="pss")
                    for half in range(2):
                        for po, s_ps in ((0, s_e), (1, s_o)):
                            rs = slice(po * HD, (po + 1) * HD)
                            nc.tensor.matmul(
                                s_ps[:, half * 512 : (half + 1) * 512],
                                KT[c][rs, kc * P : (kc + 1) * P],
                                QT[c][rs, half * 512 : (half + 1) * 512],
                                start=True,
                                stop=True,
                                tile_position=(po * HD, 0),
                            )
                    for po, s_ps, a_tiles in ((0, s_e, a_e), (1, s_o, a_o)):
                        a_t = p_a.tile(
                            [P, NTOK], BF16, tag="a", name=f"a{2 * c + po}_{kc}"
                        )
                        nc.scalar.activation(
                            a_t[:], s_ps[:], mybir.ActivationFunctionType.Exp
                        )
                        a_tiles.append(a_t)

                def emit_av_unit(h, qh, a_tiles):
                    # O^T-direct: stationary = V_aug (65 cols), moving = A^T
                    # half (N=512). Row 64 of the PSUM tile is the softmax
                    # denominator; reciprocal on DVE, partition-broadcast on
                    # the (idle) gpsimd engine, normalize on DVE straight
                    # into the O^T chunk the output projection consumes.
                    o_ps = ps_o.tile([HD + 1, 512], F32, tag="pso")
                    for kc in range(TT):
                        nc.tensor.matmul(
                            o_ps[:],
                            VA[kc][:, h * (HD + 1) : (h + 1) * (HD + 1)],
                            a_tiles[kc][:, qh * 512 : (qh + 1) * 512],
                            start=(kc == 0),
                            stop=(kc == TT - 1),
                        )
                    # the denominator row must be staged to SBUF partition 0:
                    # the custom-DVE reciprocal misreads PSUM at a partition
                    # offset (verified on HW)
                    dn = p_r.tile([1, 512], F32, tag="r")
                    nc.vector.tensor_copy(dn[:], o_ps[HD : HD + 1, :])
                    rc = p_r.tile([1, 512], F32, tag="r")
                    nc.vector.reciprocal_approx_fast(rc[:], dn[:])
                    rb = p_rb.tile([HD, 512], F32, tag="rb")
                    nc.gpsimd.partition_broadcast(rb[:], rc[:])
                    j, po = h // 2, h % 2
                    nc.vector.tensor_mul(
                        OT[
                            po * HD : (po + 1) * HD,
                            j * NTOK + qh * 512 : j * NTOK + (qh + 1) * 512,
                        ],
                        o_ps[0:HD, :],
                        rb[:],
                    )

                # prologue, ordered to match DMA arrival so the PE chases
                emit_k(0, 0)
                emit_k(0, 1)
                emit_q(0, 0)
                emit_q(0, 1)
                emit_v(0, 0)
                emit_v(0, 1)

                # steady state: per head-pair c, interleave the 8 S units
                # (exp-paced on ACT) with filler units - the V projection
                # (c=0), the next block's K/Q chains, and the previous
                # pair's AV units - so the in-order PE never stalls on exp.
                # AV units sit early in the filler order so the previous
                # pair's a-tiles free up before this pair fills the pool.
                A_PREV = None  # (a_even, a_odd) of pair c-1
                for c in range(DT):
                    fillers = []
                    if c == 0:
                        fillers += [
                            lambda tt=tt, nh=nh: emit_v(tt, nh)
                            for tt in range(1, TT)
                            for nh in range(2)
                        ]
                    nxt = []
                    if c + 1 < DT:
                        nxt = [
                            lambda cc=c + 1: emit_k(cc, 0),
                            lambda cc=c + 1: emit_q(cc, 0),
                            lambda cc=c + 1: emit_k(cc, 1),
                            lambda cc=c + 1: emit_q(cc, 1),
                        ]
                    if A_PREV is not None:
                        ae, ao = A_PREV
                        avs = [
                            lambda qh=qh, hh=2 * (c - 1) + po, at=(ae, ao)[po]: emit_av_unit(
                                hh, qh, at
                            )
                            for qh in range(2)
                            for po in range(2)
                        ]
                        fillers += nxt[:1] + avs[:2] + nxt[1:2] + avs[2:] + nxt[2:]
                    else:
                        fillers += nxt
                    a_e, a_o = [], []
                    s_units = [
                        (lambda kc=kc: emit_s_unit(c, kc, a_e, a_o)) for kc in range(TT)
                    ]
                    # round-robin: one S unit, then a spread of filler units
                    fi = 0
                    for si, s_fn in enumerate(s_units):
                        s_fn()
                        want = (si + 1) * len(fillers) // len(s_units)
                        while fi < want:
                            fillers[fi]()
                            fi += 1
                    while fi < len(fillers):
                        fillers[fi]()
                        fi += 1
                    A_PREV = (a_e, a_o)

            # ---- final AV pair interleaved with the output projection ----
            with (
                tc.tile_pool(name="p_wo", bufs=1) as p_wo,
                tc.tile_pool(name="p_y", bufs=2) as p_y,
            ):
                WO = p_wo.tile([P, DT * D], BF16, tag="wo")
                nc.sync.dma_start(WO[:, : DT * D // 2], wo_ext.ap()[:, : DT * D // 2])
                nc.scalar.dma_start(WO[:, DT * D // 2 :], wo_ext.ap()[:, DT * D // 2 :])

                ae, ao = A_PREV
                for qh in range(2):
                    emit_av_unit(2 * (DT - 1), qh, ae)
                    emit_av_unit(2 * (DT - 1) + 1, qh, ao)
                    # the qh=0 AV units complete O^T chunk 7 for q-tiles 0-3,
                    # so the output projection streams right behind them
                    for qt in range(qh * 4, qh * 4 + 4):
                        y_t = p_y.tile([P, D], F32, tag="y")
                        for nh in range(2):
                            y_ps = ps_a.tile([P, 512], F32, tag="psa")
                            for dt in range(DT):
                                nc.tensor.matmul(
                                    y_ps[:],
                                    OT[:, dt * NTOK + qt * P : dt * NTOK + (qt + 1) * P],
                                    WO[:, dt * D + nh * 512 : dt * D + (nh + 1) * 512],
                                    start=(dt == 0),
                                    stop=(dt == DT - 1),
                                )
                            nc.scalar.copy(y_t[:, nh * 512 : (nh + 1) * 512], y_ps[:])
                            nc.sync.dma_start(
                                out_ext.ap()[
                                    qt * P : (qt + 1) * P, nh * 512 : (nh + 1) * 512
                                ],
                                y_t[:, nh * 512 : (nh + 1) * 512],
                            )

    nc.compile()
    return nc


# ---------------------------------------------------------------------------
# host side
# ---------------------------------------------------------------------------

def host_prep(x, coords, is_context, rope_cache,
              ctx_in_w, ctx_in_b, ctx_out_w, ctx_out_b,
              tgt_in_w, tgt_in_b, tgt_out_w, tgt_out_b):
    x = np.asarray(x, np.float32)
    coords = np.asarray(coords, np.float32)
    is_context = np.asarray(is_context, bool)
    rope_cache = np.asarray(rope_cache, np.float32)

    keys = np.where(is_context, 0, 1).astype(np.int32)
    order = np.argsort(keys, axis=1, kind="stable")
    ctx_idx = order[:, :NCTX]
    tgt_idx = order[:, NCTX:]

    # 2D rope on host (exact fp32 mirror of the reference)
    cn = np.clip(
        coords / np.float32(IMAGE_SIZE) * np.float32(MAX_POS - 1), 0, MAX_POS - 1
    )
    y_pos = cn[..., 0].astype(np.int32)
    x_pos = cn[..., 1].astype(np.int32)
    half, quarter = D // 2, D // 4
    xr = np.empty_like(x)
    for b in range(B):
        xro = rope_cache[x_pos[b]]  # [K, quarter, 2]
        yro = rope_cache[y_pos[b]]
        xp = x[b, :, :half].reshape(K, quarter, 2)
        yp = x[b, :, half:].reshape(K, quarter, 2)
        cx, sx = xro[..., 0], xro[..., 1]
        cy, sy = yro[..., 0], yro[..., 1]
        xr[b, :, :half] = np.stack(
            [xp[..., 0] * cx - xp[..., 1] * sx, xp[..., 0] * sx + xp[..., 1] * cx], -1
        ).reshape(K, half)
        xr[b, :, half:] = np.stack(
            [yp[..., 0] * cy - yp[..., 1] * sy, yp[..., 0] * sy + yp[..., 1] * cy], -1
        ).reshape(K, half)

    def pack_dmajor(mT):
        # [din, cols] -> [P, DT*cols]: row p, col dt*cols+j = mT[dt*128+p, j]
        cols = mT.shape[1]
        return np.ascontiguousarray(
            mT.reshape(DT, P, cols).transpose(1, 0, 2).reshape(P, DT * cols)
        ).astype(ml_dtypes.bfloat16)

    def w_pack(in_w, in_b, out_w, out_b):
        w = np.array(in_w, np.float32)
        w[0:D] *= np.float32(0.125)  # fold 1/sqrt(hd) into wq (exact)
        wqkvT = np.stack(
            [pack_dmajor(np.ascontiguousarray(w[i * D : (i + 1) * D].T)) for i in range(3)]
        )
        bq = np.array(in_b[0:D], np.float32) * np.float32(0.125)
        bk = np.array(in_b[D : 2 * D], np.float32)
        biasqk = np.ascontiguousarray(
            np.concatenate([bq.reshape(DT, P).T, bk.reshape(DT, P).T], axis=1),
            np.float32,
        )
        out_w32 = np.array(out_w, np.float32)
        woT = pack_dmajor(np.ascontiguousarray(out_w32.T))
        b_total = (
            np.array(in_b[2 * D : 3 * D], np.float32) @ out_w32.T
            + np.array(out_b, np.float32)
        ).astype(np.float32)
        return wqkvT, woT, biasqk, b_total

    packs = [
        w_pack(ctx_in_w, ctx_in_b, ctx_out_w, ctx_out_b),
        w_pack(tgt_in_w, tgt_in_b, tgt_out_w, tgt_out_b),
    ]

    in_maps, scatter = [], []
    for c in range(8):
        b, role = c // 2, c % 2
        q_idx = ctx_idx[b] if role == 0 else tgt_idx[b]
        kv_idx = ctx_idx[b]
        xqT = pack_dmajor(np.ascontiguousarray(xr[b][q_idx].T))
        xkvT = pack_dmajor(np.ascontiguousarray(xr[b][kv_idx].T))
        wqkvT, woT, biasqk, b_total = packs[role]
        in_maps.append(
            {"xqT": xqT, "xkvT": xkvT, "wqkvT": wqkvT, "woT": woT, "biasqk": biasqk}
        )
        scatter.append((b, q_idx, b_total))
    return in_maps, scatter


_NC_CACHE = None


def kernel(**inputs):
    global _NC_CACHE
    in_maps, scatter = host_prep(**inputs)
    if _NC_CACHE is None:
        _NC_CACHE = build_nc()
    res = run_bass_kernel_spmd(_NC_CACHE, in_maps, core_ids=list(range(8)))
    out = np.zeros_like(np.asarray(inputs["x"], np.float32))
    for c in range(8):
        b, q_idx, b_total = scatter[c]
        out[b][q_idx] = res.results[c]["out"] + b_total
    return out


# revision 29
# speedup vs baseline: 1.0119x; 1.0119x over previous
"""Trainium2 Bass kernel for nn_CNNCrossPatchBackbone (sparse cross-patch attention).

Strategy: 8 cores = 4 batches x {ctx self-attention, tgt cross-attention}.
Each core runs an identical-shape problem: 1024 q-tokens x 1024 kv-tokens,
16 heads of dim 64, D=1024. Fully task-parallel, no collectives.

Host side does all elementwise/index prep: stable argsort, token gather, the
full 2D-RoPE rotation (exact fp32 mirror of the reference), the d-major
transpose of x, weight transposes packed [P, DT*cols] (so each tensor loads
in a few large DMAs), the 1/sqrt(hd)=2^-3 scale fold into wq/bq (exact), and
a combined output bias b_total = b_o + b_v @ Wo^T added after the gather
(softmax rows sum to 1, so the V-bias passes through attention additively).

Device side is one software-pipelined bf16 PE stream (fp32 PSUM accum):
  * Projections: Q^T/K^T in [dout, tok] layout (bias folded into the DVE
    PSUM->SBUF copy); V in natural [tok, dout] layout with a ones column
    per head (VA tiles memset to 1.0; the proj copies overwrite the data
    columns) so the AV matmul also produces the softmax denominator.
  * Scores: S^T = K_h^T^T Q^T per (head-pair, kv-tile) into [128,1024]
    PSUM; the two K=64 contractions of a head pair target disjoint PE
    row-groups via tile_position, paired by rhs column half. exp on ACT
    (max-subtraction skipped: scores ~N(0,1)); ACT runs (almost) nothing
    else, so its 222-cycle access bubble is paid only 128x on [128,1024]
    activations.
  * AV: O^T-direct - stationary = V_aug [128,65], moving = A^T (N=512),
    PSUM out [65, 512]. Row 64 is the denominator: staged to SBUF
    partition 0 on DVE (the custom-DVE reciprocal misreads PSUM at a
    partition offset - verified on HW), reciprocal_approx_fast, then
    partition-broadcast on the otherwise-idle gpsimd engine, and a DVE
    multiply writes the normalized O^T chunk directly into the single
    [128, 8*1024] O^T tile the output projection consumes.
  * Output projection accumulates y[tok, dout] from O^T chunks; biases on
    host.
  * Scheduling: the in-order PE stream interleaves, at ~0.5-2us
    granularity, each head-pair's 8 S units with the NEXT block's K/Q
    projection chains and the PREVIOUS pair's AV units, so the PE never
    stalls on the exp pipeline and the exp never starves. Inputs arrive
    in consumption-ordered quarter-tensor chunks alternating across both
    HWDGE queues (kv side first, then wv for the V-projection prologue
    fillers, then the q side). The final AV pair runs right before the
    output projection, which streams back per q-tile.
"""

import sys

sys.path.insert(0, "/opt/trn_rl_repo")

import ml_dtypes
import numpy as np

import concourse.bass as bass  # noqa: F401
import concourse.tile as tile
from concourse import bacc, mybir
from concourse.bass_utils import run_bass_kernel_spmd

B, K, D, H = 4, 2048, 1024, 16
NCTX = K // 2
NTOK = 1024  # tokens per side after the ctx/tgt split
HD = D // H  # 64
IMAGE_SIZE = 224.0
MAX_POS = 1024
P = 128
DT = D // P  # 8 d-tiles
TT = NTOK // P  # 8 token-tiles
F32 = mybir.dt.float32
BF16 = mybir.dt.bfloat16
A_BUFS = 26  # a-tile pool: ~2 head-pairs (AV frees a pair only at its last unit)


def build_nc():
    nc = bacc.Bacc("TRN2", target_bir_lowering=False, debug=False, num_devices=8)

    # all per-core tensors are packed [P, DT*cols] so each loads in 1-2 DMAs
    xq_ext = nc.dram_tensor("xqT", [P, DT * NTOK], BF16, kind="ExternalInput")
    xkv_ext = nc.dram_tensor("xkvT", [P, DT * NTOK], BF16, kind="ExternalInput")
    w_ext = nc.dram_tensor("wqkvT", [3, P, DT * D], BF16, kind="ExternalInput")
    wo_ext = nc.dram_tensor("woT", [P, DT * D], BF16, kind="ExternalInput")
    bias_ext = nc.dram_tensor("biasqk", [P, 2 * DT], F32, kind="ExternalInput")
    out_ext = nc.dram_tensor("out", [NTOK, D], F32, kind="ExternalOutput")

    with tile.TileContext(nc) as tc:
        with (
            tc.tile_pool(name="p_bias", bufs=1) as p_bias,
            tc.tile_pool(name="p_qt", bufs=DT) as p_qt,
            tc.tile_pool(name="p_kt", bufs=DT) as p_kt,
            tc.tile_pool(name="p_va", bufs=TT) as p_va,
            tc.tile_pool(name="p_a", bufs=A_BUFS) as p_a,
            tc.tile_pool(name="p_r", bufs=2) as p_r,
            tc.tile_pool(name="p_rb", bufs=2) as p_rb,
            tc.tile_pool(name="p_ot", bufs=1) as p_ot,
            tc.tile_pool(name="p_xkv", bufs=1) as p_xkv,
            tc.tile_pool(name="ps_a", bufs=2, space="PSUM") as ps_a,
            tc.tile_pool(name="ps_s", bufs=2, space="PSUM") as ps_s,
            tc.tile_pool(name="ps_o", bufs=2, space="PSUM") as ps_o,
        ):
            biasqk = p_bias.tile([P, 2 * DT], F32)
            nc.sync.dma_start(biasqk[:], bias_ext.ap())

            QT = [p_qt.tile([P, NTOK], BF16, tag="qt", name=f"qt{i}") for i in range(DT)]
            KT = [p_kt.tile([P, NTOK], BF16, tag="kt", name=f"kt{i}") for i in range(DT)]
            VA = [
                p_va.tile([P, H * (HD + 1)], BF16, tag="va", name=f"va{i}")
                for i in range(TT)
            ]
            OT = p_ot.tile([P, DT * NTOK], BF16, tag="ot")
            XKV = p_xkv.tile([P, DT * NTOK], BF16, tag="xkv")

            # ones for the V denominator columns
            for t in VA:
                nc.gpsimd.memset(t[:], 1.0)

            # ---- everything is software-pipelined into one PE stream ----
            with (
                tc.tile_pool(name="p_wv", bufs=1) as p_wv,
                tc.tile_pool(name="p_wk", bufs=1) as p_wk,
                tc.tile_pool(name="p_wq", bufs=1) as p_wq,
                tc.tile_pool(name="p_xq", bufs=1) as p_xq,
            ):
                WK = p_wk.tile([P, DT * D], BF16, tag="wk")
                WQ = p_wq.tile([P, DT * D], BF16, tag="wq")
                XQ = p_xq.tile([P, DT * NTOK], BF16, tag="xq")
                WV = p_wv.tile([P, DT * D], BF16, tag="wv")
                # consumption-ordered chunks alternating across both HWDGE
                # queues: K inputs, then the q side (Q gates the first exp,
                # so it beats wv - the V units are PE filler with slack).
                CH = DT * NTOK // 4
                groups = (
                    [(XKV, xkv_ext.ap()), (WK, w_ext.ap()[1])],
                    [(XQ, xq_ext.ap()), (WQ, w_ext.ap()[0])],
                    [(WV, w_ext.ap()[2])],
                )
                for pairs in groups:
                    for ci in range(4):
                        cs = slice(ci * CH, (ci + 1) * CH)
                        q = nc.sync if ci % 2 == 0 else nc.scalar
                        for dst, srcap in pairs:
                            q.dma_start(dst[:, cs], srcap[:, cs])

                def emit_v(tt, nh):
                    v_ps = ps_a.tile([P, 512], F32, tag="psa")
                    for dt in range(DT):
                        nc.tensor.matmul(
                            v_ps[:],
                            XKV[:, dt * NTOK + tt * P : dt * NTOK + (tt + 1) * P],
                            WV[:, dt * D + nh * 512 : dt * D + (nh + 1) * 512],
                            start=(dt == 0),
                            stop=(dt == DT - 1),
                        )
                    out_ap = VA[tt][:].rearrange("p (h c) -> p h c", c=HD + 1)[
                        :, nh * 8 : (nh + 1) * 8, 0:HD
                    ]
                    nc.vector.tensor_copy(
                        out_ap, v_ps[:].rearrange("p (h c) -> p h c", c=HD)
                    )

                def emit_k(c, nh):
                    # K^T block c (heads 2c in rows 0:64, 2c+1 in 64:128);
                    # bias folded into the PSUM->SBUF copy
                    ps = ps_a.tile([P, 512], F32, tag="psa")
                    for dt in range(DT):
                        nc.tensor.matmul(
                            ps[:],
                            WK[:, dt * D + c * P : dt * D + (c + 1) * P],
                            XKV[:, dt * NTOK + nh * 512 : dt * NTOK + (nh + 1) * 512],
                            start=(dt == 0),
                            stop=(dt == DT - 1),
                        )
                    nc.vector.tensor_scalar_add(
                        KT[c][:, nh * 512 : (nh + 1) * 512],
                        ps[:],
                        biasqk[:, DT + c : DT + c + 1],
                    )

                def emit_q(c, nh):
                    ps = ps_a.tile([P, 512], F32, tag="psa")
                    for dt in range(DT):
                        nc.tensor.matmul(
                            ps[:],
                            WQ[:, dt * D + c * P : dt * D + (c + 1) * P],
                            XQ[:, dt * NTOK + nh * 512 : dt * NTOK + (nh + 1) * 512],
                            start=(dt == 0),
                            stop=(dt == DT - 1),
                        )
                    nc.vector.tensor_scalar_add(
                        QT[c][:, nh * 512 : (nh + 1) * 512], ps[:], biasqk[:, c : c + 1]
                    )

                def emit_s_unit(c, kc, a_e, a_o):
                    # scores for BOTH heads of pair c against kv-tile kc: the
                    # two K=64 contractions occupy disjoint row-groups of the
                    # PE array (tile_position). Pair the matmuls BY COLUMN
                    # HALF so consecutive instructions stream the same rhs
                    # columns and co-stream through the array on HW.
                    s_e = ps_s.tile([P, NTOK], F32, tag="pss")
                    s_o = ps_s.tile([P, NTOK], F32, tag="pss")
                    for half in range(2):
                        for po, s_ps in ((0, s_e), (1, s_o)):
                            rs = slice(po * HD, (po + 1) * HD)
                            nc.tensor.matmul(
                                s_ps[:, half * 512 : (half + 1) * 512],
                                KT[c][rs, kc * P : (kc + 1) * P],
                                QT[c][rs, half * 512 : (half + 1) * 512],
                                start=True,
                                stop=True,
                                tile_position=(po * HD, 0),
                            )
                    for po, s_ps, a_tiles in ((0, s_e, a_e), (1, s_o, a_o)):
                        a_t = p_a.tile(
                            [P, NTOK], BF16, tag="a", name=f"a{2 * c + po}_{kc}"
                        )
                        nc.scalar.activation(
                            a_t[:], s_ps[:], mybir.ActivationFunctionType.Exp
                        )
                        a_tiles.append(a_t)

                def emit_av_unit(h, qh, a_tiles):
                    # O^T-direct: stationary = V_aug (65 cols), moving = A^T
                    # half (N=512). Row 64 of the PSUM tile is the softmax
                    # denominator; reciprocal on DVE, partition-broadcast on
                    # the (idle) gpsimd engine, normalize on DVE straight
                    # into the O^T chunk the output projection consumes.
                    o_ps = ps_o.tile([HD + 1, 512], F32, tag="pso")
                    for kc in range(TT):
                        nc.tensor.matmul(
                            o_ps[:],
                            VA[kc][:, h * (HD + 1) : (h + 1) * (HD + 1)],
                            a_tiles[kc][:, qh * 512 : (qh + 1) * 512],
                            start=(kc == 0),
                            stop=(kc == TT - 1),
                        )
                    # the denominator row must be staged to SBUF partition 0:
                    # the custom-DVE reciprocal misreads PSUM at a partition
                    # offset (verified on HW)
                    dn = p_r.tile([1, 512], F32, tag="r")
                    nc.vector.tensor_copy(dn[:], o_ps[HD : HD + 1, :])
                    rc = p_r.tile([1, 512], F32, tag="r")
                    nc.vector.reciprocal_approx_fast(rc[:], dn[:])
                    rb = p_rb.tile([HD, 512], F32, tag="rb")
                    nc.gpsimd.partition_broadcast(rb[:], rc[:])
                    j, po = h // 2, h % 2
                    nc.vector.tensor_mul(
                        OT[
                            po * HD : (po + 1) * HD,
                            j * NTOK + qh * 512 : j * NTOK + (qh + 1) * 512,
                        ],
                        o_ps[0:HD, :],
                        rb[:],
                    )

                # prologue, ordered to match DMA arrival so the PE chases;
                # no V here - wv lands last, and the V units are c=0 fillers
                emit_k(0, 0)
                emit_k(0, 1)
                emit_q(0, 0)
                emit_q(0, 1)

                # steady state: per head-pair c, interleave the 8 S units
                # (exp-paced on ACT) with filler units - the V projection
                # (c=0), the next block's K/Q chains, and the previous
                # pair's AV units - so the in-order PE never stalls on exp.
                # AV units sit early in the filler order so the previous
                # pair's a-tiles free up before this pair fills the pool.
                A_PREV = None  # (a_even, a_odd) of pair c-1
                for c in range(DT):
                    fillers = []
                    if c == 0:
                        fillers += [
                            lambda tt=tt, nh=nh: emit_v(tt, nh)
                            for tt in range(TT)
                            for nh in range(2)
                        ]
                    nxt = []
                    if c + 1 < DT:
                        nxt = [
                            lambda cc=c + 1: emit_k(cc, 0),
                            lambda cc=c + 1: emit_q(cc, 0),
                            lambda cc=c + 1: emit_k(cc, 1),
                            lambda cc=c + 1: emit_q(cc, 1),
                        ]
                    if A_PREV is not None:
                        ae, ao = A_PREV
                        avs = [
                            lambda qh=qh, hh=2 * (c - 1) + po, at=(ae, ao)[po]: emit_av_unit(
                                hh, qh, at
                            )
                            for qh in range(2)
                            for po in range(2)
                        ]
                        fillers += nxt[:1] + avs[:2] + nxt[1:2] + avs[2:] + nxt[2:]
                    else:
                        fillers += nxt
                    a_e, a_o = [], []
                    s_units = [
                        (lambda kc=kc: emit_s_unit(c, kc, a_e, a_o)) for kc in range(TT)
                    ]
                    # round-robin: one S unit, then a spread of filler units
                    fi = 0
                    for si, s_fn in enumerate(s_units):
                        s_fn()
                        want = (si + 1) * len(fillers) // len(s_units)
                        while fi < want:
                            fillers[fi]()
                            fi += 1
                    while fi < len(fillers):
                        fillers[fi]()
                        fi += 1
                    A_PREV = (a_e, a_o)

            # ---- final AV pair interleaved with the output projection ----
            with (
                tc.tile_pool(name="p_wo", bufs=1) as p_wo,
                tc.tile_pool(name="p_y", bufs=2) as p_y,
            ):
                WO = p_wo.tile([P, DT * D], BF16, tag="wo")
                nc.sync.dma_start(WO[:, : DT * D // 2], wo_ext.ap()[:, : DT * D // 2])
                nc.scalar.dma_start(WO[:, DT * D // 2 :], wo_ext.ap()[:, DT * D // 2 :])

                ae, ao = A_PREV
                for qh in range(2):
                    emit_av_unit(2 * (DT - 1), qh, ae)
                    emit_av_unit(2 * (DT - 1) + 1, qh, ao)
                    # the qh=0 AV units complete O^T chunk 7 for q-tiles 0-3,
                    # so the output projection streams right behind them
                    for qt in range(qh * 4, qh * 4 + 4):
                        y_t = p_y.tile([P, D], F32, tag="y")
                        for nh in range(2):
                            y_ps = ps_a.tile([P, 512], F32, tag="psa")
                            for dt in range(DT):
                                nc.tensor.matmul(
                                    y_ps[:],
                                    OT[:, dt * NTOK + qt * P : dt * NTOK + (qt + 1) * P],
                                    WO[:, dt * D + nh * 512 : dt * D + (nh + 1) * 512],
                                    start=(dt == 0),
                                    stop=(dt == DT - 1),
                                )
                            nc.scalar.copy(y_t[:, nh * 512 : (nh + 1) * 512], y_ps[:])
                            nc.sync.dma_start(
                                out_ext.ap()[
                                    qt * P : (qt + 1) * P, nh * 512 : (nh + 1) * 512
                                ],
                                y_t[:, nh * 512 : (nh + 1) * 512],
                            )

    nc.compile()
    return nc


# ---------------------------------------------------------------------------
# host side
# ---------------------------------------------------------------------------

def host_prep(x, coords, is_context, rope_cache,
              ctx_in_w, ctx_in_b, ctx_out_w, ctx_out_b,
              tgt_in_w, tgt_in_b, tgt_out_w, tgt_out_b):
    x = np.asarray(x, np.float32)
    coords = np.asarray(coords, np.float32)
    is_context = np.asarray(is_context, bool)
    rope_cache = np.asarray(rope_cache, np.float32)

    keys = np.where(is_context, 0, 1).astype(np.int32)
    order = np.argsort(keys, axis=1, kind="stable")
    ctx_idx = order[:, :NCTX]
    tgt_idx = order[:, NCTX:]

    # 2D rope on host (exact fp32 mirror of the reference)
    cn = np.clip(
        coords / np.float32(IMAGE_SIZE) * np.float32(MAX_POS - 1), 0, MAX_POS - 1
    )
    y_pos = cn[..., 0].astype(np.int32)
    x_pos = cn[..., 1].astype(np.int32)
    half, quarter = D // 2, D // 4
    xr = np.empty_like(x)
    for b in range(B):
        xro = rope_cache[x_pos[b]]  # [K, quarter, 2]
        yro = rope_cache[y_pos[b]]
        xp = x[b, :, :half].reshape(K, quarter, 2)
        yp = x[b, :, half:].reshape(K, quarter, 2)
        cx, sx = xro[..., 0], xro[..., 1]
        cy, sy = yro[..., 0], yro[..., 1]
        xr[b, :, :half] = np.stack(
            [xp[..., 0] * cx - xp[..., 1] * sx, xp[..., 0] * sx + xp[..., 1] * cx], -1
        ).reshape(K, half)
        xr[b, :, half:] = np.stack(
            [yp[..., 0] * cy - yp[..., 1] * sy, yp[..., 0] * sy + yp[..., 1] * cy], -1
        ).reshape(K, half)

    def pack_dmajor(mT):
        # [din, cols] -> [P, DT*cols]: row p, col dt*cols+j = mT[dt*128+p, j]
        cols = mT.shape[1]
        return np.ascontiguousarray(
            mT.reshape(DT, P, cols).transpose(1, 0, 2).reshape(P, DT * cols)
        ).astype(ml_dtypes.bfloat16)

    def w_pack(in_w, in_b, out_w, out_b):
        w = np.array(in_w, np.float32)
        w[0:D] *= np.float32(0.125)  # fold 1/sqrt(hd) into wq (exact)
        wqkvT = np.stack(
            [pack_dmajor(np.ascontiguousarray(w[i * D : (i + 1) * D].T)) for i in range(3)]
        )
        bq = np.array(in_b[0:D], np.float32) * np.float32(0.125)
        bk = np.array(in_b[D : 2 * D], np.float32)
        biasqk = np.ascontiguousarray(
            np.concatenate([bq.reshape(DT, P).T, bk.reshape(DT, P).T], axis=1),
            np.float32,
        )
        out_w32 = np.array(out_w, np.float32)
        woT = pack_dmajor(np.ascontiguousarray(out_w32.T))
        b_total = (
            np.array(in_b[2 * D : 3 * D], np.float32) @ out_w32.T
            + np.array(out_b, np.float32)
        ).astype(np.float32)
        return wqkvT, woT, biasqk, b_total

    packs = [
        w_pack(ctx_in_w, ctx_in_b, ctx_out_w, ctx_out_b),
        w_pack(tgt_in_w, tgt_in_b, tgt_out_w, tgt_out_b),
    ]

    in_maps, scatter = [], []
    for c in range(8):
        b, role = c // 2, c % 2
        q_idx = ctx_idx[b] if role == 0 else tgt_idx[b]
        kv_idx = ctx_idx[b]
        xqT = pack_dmajor(np.ascontiguousarray(xr[b][q_idx].T))
        xkvT = pack_dmajor(np.ascontiguousarray(xr[b][kv_idx].T))
        wqkvT, woT, biasqk, b_total = packs[role]
        in_maps.append(
            {"xqT": xqT, "xkvT": xkvT, "wqkvT": wqkvT, "woT": woT, "biasqk": biasqk}
        )
        scatter.append((b, q_idx, b_total))
    return in_maps, scatter


_NC_CACHE = None


def kernel(**inputs):
    global _NC_CACHE
    in_maps, scatter = host_prep(**inputs)
    if _NC_CACHE is None:
        _NC_CACHE = build_nc()
    res = run_bass_kernel_spmd(_NC_CACHE, in_maps, core_ids=list(range(8)))
    out = np.zeros_like(np.asarray(inputs["x"], np.float32))
    for c in range(8):
        b, q_idx, b_total = scatter[c]
        out[b][q_idx] = res.results[c]["out"] + b_total
    return out
